# revision 20
# baseline (speedup 1.0000x reference)
"""FlowNetC correlation on Trainium2 — V3: 4x(4y x 8x) quad chunks, Q=192,
col-tiled M=32 matmuls (tile_position (0, 32g)). See kernel_v2.py docstring.
"""
import sys
sys.path.insert(0, '/opt/trn_rl_repo')
from contextlib import ExitStack
import numpy as np
import ml_dtypes

import concourse.bass as bass
import concourse.mybir as mybir
from concourse import bacc
from concourse.tile import TileContext
from concourse.bass_utils import run_bass_kernel_spmd

AP = bass.AP
C = 128; H = 128; W = 256
HY, HX = 12, 16                # halo of a 4x8 sub-chunk
NCY, NCX = H // 16, W // 8     # 8 row-blocks (16 rows each), 32 col-chunks
QN = HY * HX                   # 192
Hp, Wp = H + 8, W + 8

_CACHED = {}


def _build_kernel(reps=1):
    nc = bacc.Bacc("TRN2", target_bir_lowering=False, debug=False)
    NCH = NCY * NCX
    a = nc.dram_tensor("a", [C, NCH, 128], mybir.dt.bfloat16, kind="ExternalInput")
    b = nc.dram_tensor("b", [C, Hp * Wp], mybir.dt.bfloat16, kind="ExternalInput")
    o = nc.dram_tensor("o", [NCY, 128, NCX * QN], mybir.dt.bfloat16,
                       kind="ExternalOutput")
    with TileContext(nc) as tc:
        with ExitStack() as ctx:
            const = ctx.enter_context(tc.tile_pool(name="const", bufs=1))
            apool = ctx.enter_context(tc.tile_pool(name="apool", bufs=4))
            wpool = ctx.enter_context(tc.tile_pool(name="wpool", bufs=3))
            ps = ctx.enter_context(tc.tile_pool(name="ps", bufs=8, space="PSUM"))

            b_sb = const.tile([C, Hp * Wp], mybir.dt.bfloat16)
            nc.sync.dma_start(out=b_sb[:], in_=b[:])

            if reps > 1:
                ctx.enter_context(tc.For_i(0, reps, 1,
                                           hint_engines=(mybir.EngineType.PE,)))
            for cy in range(NCY):
                # half-row a tiles: the first half-load unblocks PE sooner
                # after the loop back-edge barrier
                a_sbs = []
                for h in range(2):
                    a_sb = apool.tile([C, NCX * 64], mybir.dt.bfloat16, tag="a_h")
                    lo = cy * NCX + h * (NCX // 2)
                    nc.sync.dma_start(out=a_sb[:], in_=a[:, lo:lo + NCX // 2, :]
                                      .rearrange("c n p -> c (n p)"))
                    a_sbs.append(a_sb)
                w_row = wpool.tile([128, NCX * QN], mybir.dt.bfloat16)
                for cx in range(NCX):
                    y0, x0 = cy * 16, cx * 8
                    a_sb = a_sbs[cx // (NCX // 2)]
                    ci = cx % (NCX // 2)
                    g_ps = ps.tile([128, QN], mybir.dt.float32)
                    for g in range(4):
                        bh = AP(tensor=b_sb.tensor, offset=(y0 + 4 * g) * Wp + x0,
                                ap=[[Hp * Wp, C], [Wp, HY], [1, HX]])
                        nc.tensor.matmul(
                            g_ps[32 * g:32 * (g + 1), :],
                            a_sb[:, ci * 128 + 32 * g:ci * 128 + 32 * (g + 1)],
                            bh, start=True, stop=True,
                            tile_position=(0, 32 * g))
                    if cx % 2 == 0:
                        nc.vector.tensor_copy(w_row[:, cx * QN:(cx + 1) * QN], g_ps[:])
                    else:
                        nc.scalar.copy(w_row[:, cx * QN:(cx + 1) * QN], g_ps[:])
                # split the store: halves start draining before the full row
                # is copied (shorter pipeline tail at the loop back-edge)
                half = NCX * QN // 2
                for h in range(2):
                    odst = AP(tensor=o, offset=cy * 128 * NCX * QN + h * half,
                              ap=[[NCX * QN, 128], [1, half]])
                    nc.gpsimd.dma_start(out=odst, in_=w_row[:, h * half:(h + 1) * half])
    nc.compile()
    return nc


def _prep_inputs(input1, input2):
    a = (input1 * (1.0 / C)).astype(ml_dtypes.bfloat16)
    # a[c, chunk=(cy,cx), p=32*g+8*ty+tx]
    a = a.reshape(C, NCY, 4, 4, NCX, 8).transpose(0, 1, 4, 2, 3, 5).reshape(
        C, NCY * NCX, 128)
    bp = np.zeros((C, Hp, Wp), dtype=ml_dtypes.bfloat16)
    bp[:, 4:4 + H, 4:4 + W] = input2.astype(ml_dtypes.bfloat16)
    return {"a": np.ascontiguousarray(a), "b": bp.reshape(C, Hp * Wp)}


def _finish_output(o_np):
    """o[cy, 32*g+8*ty+tx, cx*192 + 16*ty+tx + 16*dy+dx] -> [81, H, W] fp32."""
    o_np = np.ascontiguousarray(o_np)
    scy, sp, sq = o_np.strides
    v = np.lib.stride_tricks.as_strided(
        o_np,
        shape=(NCY, 4, 4, 8, NCX, 9, 9),
        strides=(scy, 32 * sp, 8 * sp + 16 * sq, sp + sq, QN * sq, 16 * sq, sq))
    t = v.transpose(5, 6, 0, 1, 2, 4, 3).astype(np.float32)
    return t.reshape(81, H, W)


def kernel(input1, input2):
    input1 = np.asarray(input1, dtype=np.float32)
    input2 = np.asarray(input2, dtype=np.float32)
    B = input1.shape[0]
    assert input1.shape == (B, C, H, W) and input2.shape == (B, C, H, W)
    if "nc" not in _CACHED:
        _CACHED["nc"] = _build_kernel()
    nc = _CACHED["nc"]
    in_maps = [_prep_inputs(input1[b], input2[b]) for b in range(B)]
    res = run_bass_kernel_spmd(nc, in_maps, list(range(B)))
    return np.stack([_finish_output(res.results[b]["o"]) for b in range(B)])
